# revision 36
# baseline (speedup 1.0000x reference)
"""Trainium2 Bass kernel for nn_LCN (locally-connected network).

Computation:
  x: (512, 1, 280, 280) -> non-overlapping 28x28 patches (10x10 grid, P=100)
  y[b, f, p] = sum_q x[b, p, q] * w[f*100+p, q]    (q = k*28+l, 784 per patch)
  y = relu(y + bias[f*100+p]);  out = y_flat @ dec_w.T + dec_b  (j = f*100+p)

Sharding: patch-parallel, perfectly balanced. All cores run the same
program on 12 full patches (all 512 images) plus one HALF patch (one
256-image half of a shared patch; patches 96-99 are each split between two
cores). Per core:
  - host stages x TRANSPOSED as xT [128 px, chunk, batch] bf16 (im2col +
    transpose + cast on host; DMA reads are contiguous multi-KB runs per
    partition at full HBM bandwidth)
  - conv: one matmul per (128-px chunk, patch-pair 32-col PSUM window),
    K=128 N=512 bf16 accumulating per patch group; the half patch uses
    N=256 chunks into its own small PSUM tile
  - ACT: relu(psum + bias) -> y_sb (bf16)
  - decoder: accumulating matmuls into one [10, 512] PSUM tile; the half
    patch adds only into its 256 image columns
Host sums the 8 per-core partial decoder outputs and adds dec_b.
"""

import sys

import numpy as np

for _p in ("/opt/trn_rl_repo", "/opt/trn_rl_repo/concourse"):
    if _p not in sys.path:
        sys.path.insert(0, _p)

import concourse.mybir as mybir
import concourse.tile as tile
from concourse import bacc

F32 = mybir.dt.float32
BF16 = mybir.dt.bfloat16

# Problem constants
B = 512
HB = 256             # half batch (shared-patch image half)
P = 100
F = 16
OUT = 10
PPX = 784            # pixels per patch (28*28)
NCORES = 8

NPF = 12             # full patches per core
NCF = 74             # full-region chunks: ceil(12*784/128) (64 px end pad)
NCH = 7              # half-region chunks: ceil(784/128) (112 px end pad)
# full-region x splits; the half region is one final split
_SPLIT_SIZES = [4, 8, 12, 12, 12, 12, 14]
DMA_SPLITS = []
_c = 0
for _s in _SPLIT_SIZES:
    DMA_SPLITS.append((_c, _s))
    _c += _s
assert _c == NCF


def conv_plan():
    """Full-region matmul plan: one entry per (chunk, patch-pair window)."""
    plan = []
    for t in range(NCF):
        p0 = min((128 * t) // PPX, NPF - 1)
        p1 = min((128 * t + 127) // PPX, NPF - 1)
        g = p0 // 8
        pairs = sorted({(p - 8 * g) // 2 for p in (p0, p1)})
        for k in pairs:
            first = ((8 * g + 2 * k) * PPX) // 128
            last = min(((8 * g + 2 * k + 2) * PPX - 1) // 128, NCF - 1)
            plan.append((t, g, k, t == first, t == last))
    return plan

PLAN = conv_plan()
NMM = len(PLAN)
NMM_TOT = NMM + NCH  # plus the half-patch tiles
ROWS = [16 * 8, 16 * 4]  # used psum rows per full group


def build_program():
    nc = bacc.Bacc("TRN2")
    x_d = nc.dram_tensor("x", [128, NCF * B + NCH * HB], BF16,
                         kind="ExternalInput")
    # full w tiles, then half-patch w tiles, then 3 decoder groups
    w_d = nc.dram_tensor("w", [128, NMM_TOT * 32 + 3 * OUT], BF16,
                         kind="ExternalInput")
    b_d = nc.dram_tensor("bias", [128, 3], F32, kind="ExternalInput")
    o_d = nc.dram_tensor("out", [OUT, B], F32, kind="ExternalOutput")
    o2_d = nc.dram_tensor("out2", [OUT, HB], F32, kind="ExternalOutput")

    with tile.TileContext(nc) as tc:
        with (
            tc.tile_pool(name="const", bufs=1) as constp,
            tc.tile_pool(name="yps", bufs=2, space="PSUM") as ypsp,
            tc.tile_pool(name="hps", bufs=1, space="PSUM") as hpsp,
            tc.tile_pool(name="wps", bufs=1, space="PSUM") as wpsp,
            tc.tile_pool(name="ops", bufs=1, space="PSUM") as opsp,
        ):
            w_sb = constp.tile([128, NMM_TOT * 32 + 3 * OUT], BF16)
            nc.scalar.dma_start(out=w_sb[:], in_=w_d[:])
            dec_sb = w_sb[:, NMM_TOT * 32:]
            bias_sb = constp.tile([128, 3], F32)
            nc.scalar.dma_start(out=bias_sb[:], in_=b_d[:])

            xt = constp.tile([128, NCF, B], BF16)
            for c0, n in DMA_SPLITS:
                nc.sync.dma_start(
                    out=xt[:, c0:c0 + n, :],
                    in_=x_d[:, c0 * B:(c0 + n) * B],
                )
            xh = constp.tile([128, NCH, HB], BF16)
            nc.sync.dma_start(
                out=xh[:],
                in_=x_d[:, NCF * B:NCF * B + NCH * HB],
            )

            # PE clock warm-up during the DMA pipe-fill window: M=128 dummy
            # matmuls on a memset scratch tile (output never read)
            warm_sb = constp.tile([128, 128 + B], BF16)
            nc.gpsimd.memset(warm_sb[:], 0.0)
            warm_ps = wpsp.tile([128, B], F32)
            for _ in range(9):
                nc.tensor.matmul(
                    warm_ps[:],
                    warm_sb[:, 0:128],
                    warm_sb[:, 128:],
                    start=True,
                    stop=True,
                )

            y_sb = constp.tile([128, 2, B], BF16)
            yh_sb = constp.tile([16, HB], BF16)
            ps = [ypsp.tile([128, B], F32, name=f"ps{g}") for g in range(2)]
            ps_h = hpsp.tile([32, HB], F32)
            out_ps = opsp.tile([OUT, B], F32)
            out_ps_h = hpsp.tile([OUT, HB], F32)
            out_sb = constp.tile([OUT, B], F32)
            out_h_sb = constp.tile([OUT, HB], F32)

            def conv_mm(i, t, g, k, st, sp):
                nc.tensor.matmul(
                    ps[g][32 * k:32 * k + 32, :],
                    w_sb[:, 32 * i:32 * i + 32],
                    xt[:, t, :],
                    start=st,
                    stop=sp,
                    tile_position=(0, 32 * k),
                )

            ga = [e for e in enumerate(PLAN) if e[1][1] == 0]
            gb = [e for e in enumerate(PLAN) if e[1][1] == 1]
            for i, (t, _, k, st, sp) in ga:
                conv_mm(i, t, 0, k, st, sp)
            nc.scalar.activation(
                out=y_sb[0:ROWS[0], 0, :],
                in_=ps[0][0:ROWS[0], :],
                func=mybir.ActivationFunctionType.Relu,
                bias=bias_sb[0:ROWS[0], 0:1],
            )
            # a few group-B matmuls before the group-A decode so the PE
            # FIFO doesn't stall waiting on the relu
            for i, (t, _, k, st, sp) in gb[:8]:
                conv_mm(i, t, 1, k, st, sp)
            nc.tensor.matmul(
                out_ps[:],
                dec_sb[0:ROWS[0], 0:OUT],
                y_sb[0:ROWS[0], 0, :],
                start=True,
                stop=False,
            )
            for i, (t, _, k, st, sp) in gb[8:]:
                conv_mm(i, t, 1, k, st, sp)
            nc.scalar.activation(
                out=y_sb[0:ROWS[1], 1, :],
                in_=ps[1][0:ROWS[1], :],
                func=mybir.ActivationFunctionType.Relu,
                bias=bias_sb[0:ROWS[1], 1:2],
            )
            # half patch: 7 N=256 chunks into its own psum, then its decode
            # lands only in this core's image-half columns of out_ps
            for th in range(NCH):
                nc.tensor.matmul(
                    ps_h[:],
                    w_sb[:, 32 * (NMM + th):32 * (NMM + th) + 32],
                    xh[:, th, :],
                    start=(th == 0),
                    stop=(th == NCH - 1),
                )
            nc.scalar.activation(
                out=yh_sb[:],
                in_=ps_h[0:16, :],
                func=mybir.ActivationFunctionType.Relu,
                bias=bias_sb[0:16, 2:3],
            )
            nc.tensor.matmul(
                out_ps[:],
                dec_sb[0:ROWS[1], OUT:2 * OUT],
                y_sb[0:ROWS[1], 1, :],
                start=False,
                stop=True,
            )
            nc.tensor.matmul(
                out_ps_h[:],
                dec_sb[0:16, 2 * OUT:3 * OUT],
                yh_sb[:],
                start=True,
                stop=True,
            )
            nc.vector.tensor_copy(out_sb[:], out_ps[:])
            nc.sync.dma_start(out=o_d[:], in_=out_sb[:])
            nc.vector.tensor_copy(out_h_sb[:], out_ps_h[:])
            nc.sync.dma_start(out=o2_d[:], in_=out_h_sb[:])

    return nc


def stage_core(core, x_pm, weight, bias, dec_w):
    """Host-side staging for one core. x_pm: (B, 100, 784) float32."""
    import ml_dtypes

    p0 = NPF * core
    pids = list(range(p0, p0 + NPF))
    pid_h = 96 + core // 2
    img_off = (core % 2) * HB

    xs = np.zeros((B, NCF * 128), np.float32)
    xs[:, :NPF * PPX] = x_pm[:, p0:p0 + NPF, :].reshape(B, NPF * PPX)
    xs = np.ascontiguousarray(
        xs.reshape(B, NCF, 128).transpose(2, 1, 0)
    ).reshape(128, NCF * B)
    xhs = np.zeros((HB, NCH * 128), np.float32)
    xhs[:, :PPX] = x_pm[img_off:img_off + HB, pid_h, :]
    xhs = np.ascontiguousarray(
        xhs.reshape(HB, NCH, 128).transpose(2, 1, 0)
    ).reshape(128, NCH * HB)
    x_all = np.concatenate([xs, xhs], axis=1).astype(ml_dtypes.bfloat16)

    wr = np.asarray(weight, np.float32).reshape(F, P, PPX)
    w_big = np.zeros((128, NMM_TOT * 32), np.float32)
    for i, (t, g, k, _, _) in enumerate(PLAN):
        for r in range(128):
            px = 128 * t + r
            p = px // PPX
            if p >= NPF:
                continue
            pl = p - 8 * g
            if pl < 0 or pl // 2 != k:
                continue
            q = px % PPX
            w_big[r, 32 * i + (pl % 2) * 16:32 * i + (pl % 2) * 16 + F] = \
                wr[:, p0 + p, q]
    for th in range(NCH):
        for r in range(128):
            q = 128 * th + r
            if q >= PPX:
                continue
            i = NMM + th
            w_big[r, 32 * i:32 * i + F] = wr[:, pid_h, q]

    br = np.asarray(bias, np.float32).reshape(F, P)
    dr = np.asarray(dec_w, np.float32).reshape(OUT, F, P)
    b_st = np.zeros((128, 3), np.float32)
    d_st = np.zeros((128, 3 * OUT), np.float32)
    for p in range(NPF):
        g, pl = p // 8, p % 8
        j = 16 * pl + np.arange(F)
        b_st[j, g] = br[:, pids[p]]
        d_st[j[:, None], g * OUT + np.arange(OUT)[None, :]] = \
            dr[:, :, pids[p]].T
    j = np.arange(F)
    b_st[j, 2] = br[:, pid_h]
    d_st[j[:, None], 2 * OUT + np.arange(OUT)[None, :]] = dr[:, :, pid_h].T

    w_all = np.concatenate([w_big, d_st], axis=1).astype(ml_dtypes.bfloat16)
    return {"x": x_all, "w": w_all, "bias": b_st}


_cache = {}


def _get_nc():
    if "nc" not in _cache:
        nc = build_program()
        nc.finalize()
        _cache["nc"] = nc
    return _cache["nc"]


def make_in_maps(x, weight, bias, dec_w):
    x = np.asarray(x, np.float32)
    # patch-major pixel order: (b, ph, pw, k, l)
    x_pm = np.ascontiguousarray(
        x.reshape(B, 10, 28, 10, 28).transpose(0, 1, 3, 2, 4)
    ).reshape(B, P, PPX)
    return [stage_core(c, x_pm, weight, bias, dec_w) for c in range(NCORES)]


def combine(results, dec_b):
    acc = np.zeros((OUT, B), np.float32)
    for core, r in enumerate(results):
        acc += r["out"]
        off = (core % 2) * HB
        acc[:, off:off + HB] += r["out2"]
    return acc.T + np.asarray(dec_b, np.float32)


def _install_ntff_hook():
    """Provide the missing antenv.axon_hooks module so trace=True works
    under axon (replicates trn_boot._ntff_profile_via_ctypes)."""
    import contextlib
    import ctypes
    import types

    if "antenv.axon_hooks" in sys.modules:
        return
    so_path = "/opt/axon/libaxon_pjrt.so"
    holder = {}
    mod = types.ModuleType("antenv.axon_hooks")
    mod.set_axon_ntff_profile_hook = lambda h: holder.__setitem__("h", h)
    mod.get_axon_ntff_profile_hook = lambda: holder.get("h")
    sys.modules["antenv.axon_hooks"] = mod
    try:
        import antenv
        antenv.axon_hooks = mod
    except ImportError:
        pass

    lib = ctypes.CDLL(so_path)
    if not hasattr(lib, "axon_start_nrt_profile"):
        return
    lib.axon_start_nrt_profile.argtypes = [
        ctypes.POINTER(ctypes.c_int64), ctypes.c_size_t]
    lib.axon_start_nrt_profile.restype = ctypes.c_int64
    lib.axon_stop_nrt_profile.argtypes = [ctypes.c_char_p]
    lib.axon_stop_nrt_profile.restype = ctypes.c_int64

    @contextlib.contextmanager
    def _hook(output_dir, device_ids):
        import jax
        jax.devices()
        if device_ids:
            ids = (ctypes.c_int64 * len(device_ids))(*device_ids)
            rc = lib.axon_start_nrt_profile(ids, len(device_ids))
        else:
            rc = lib.axon_start_nrt_profile(None, 0)
        if rc != 0:
            raise RuntimeError(f"axon_start_nrt_profile rc={rc}")
        try:
            yield
        finally:
            n = lib.axon_stop_nrt_profile(str(output_dir).encode())
            print(f"profile: {n} file(s) written to {output_dir}")

    mod.set_axon_ntff_profile_hook(_hook)


def run(x, weight, bias, dec_w, dec_b, trace=False):
    from concourse import bass_utils
    from concourse.bass_utils import run_bass_kernel_spmd

    if trace:
        _install_ntff_hook()
        bass_utils.upload_artifacts = lambda tmpdir: tmpdir

    nc = _get_nc()
    in_maps = make_in_maps(x, weight, bias, dec_w)
    r = run_bass_kernel_spmd(nc, in_maps, list(range(NCORES)), trace=trace)
    return combine(r.results, dec_b), r


def kernel(x, weight, bias, dec_w, dec_b):
    out, _ = run(x, weight, bias, dec_w, dec_b, trace=False)
    return out


# revision 37
# speedup vs baseline: 1.1255x; 1.1255x over previous
"""Trainium2 Bass kernel for nn_LCN (locally-connected network).

Computation:
  x: (512, 1, 280, 280) -> non-overlapping 28x28 patches (10x10 grid, P=100)
  y[b, f, p] = sum_q x[b, p, q] * w[f*100+p, q]    (q = k*28+l, 784 per patch)
  y = relu(y + bias[f*100+p]);  out = y_flat @ dec_w.T + dec_b  (j = f*100+p)

Sharding: patch-parallel, perfectly balanced. All cores run the same
program on 12 full patches (all 512 images) plus one HALF patch (one
256-image half of a shared patch; patches 96-99 are each split between two
cores). Per core:
  - host stages x TRANSPOSED as xT [128 px, chunk, batch] bf16 (im2col +
    transpose + cast on host; DMA reads are contiguous multi-KB runs per
    partition at full HBM bandwidth)
  - conv: one matmul per (128-px chunk, patch-pair 32-col PSUM window),
    K=128 N=512 bf16 accumulating per patch group; the half patch uses
    N=256 chunks into its own small PSUM tile
  - ACT: relu(psum + bias) -> y_sb (bf16)
  - decoder: accumulating matmuls into one [10, 512] PSUM tile; the half
    patch adds only into its 256 image columns
Host sums the 8 per-core partial decoder outputs and adds dec_b.
"""

import sys

import numpy as np

for _p in ("/opt/trn_rl_repo", "/opt/trn_rl_repo/concourse"):
    if _p not in sys.path:
        sys.path.insert(0, _p)

import concourse.mybir as mybir
import concourse.tile as tile
from concourse import bacc

F32 = mybir.dt.float32
BF16 = mybir.dt.bfloat16

# Problem constants
B = 512
HB = 256             # half batch (shared-patch image half)
P = 100
F = 16
OUT = 10
PPX = 784            # pixels per patch (28*28)
NCORES = 8

NPF = 12             # full patches per core
NCF = 74             # full-region chunks: ceil(12*784/128) (64 px end pad)
NCH = 7              # half-region chunks: ceil(784/128) (112 px end pad)
# full-region x splits; the half region is one final split
_SPLIT_SIZES = [4, 8, 12, 12, 12, 12, 14]
DMA_SPLITS = []
_c = 0
for _s in _SPLIT_SIZES:
    DMA_SPLITS.append((_c, _s))
    _c += _s
assert _c == NCF


def conv_plan():
    """Full-region matmul plan: one entry per (chunk, patch-pair window)."""
    plan = []
    for t in range(NCF):
        p0 = min((128 * t) // PPX, NPF - 1)
        p1 = min((128 * t + 127) // PPX, NPF - 1)
        g = p0 // 8
        pairs = sorted({(p - 8 * g) // 2 for p in (p0, p1)})
        for k in pairs:
            first = ((8 * g + 2 * k) * PPX) // 128
            last = min(((8 * g + 2 * k + 2) * PPX - 1) // 128, NCF - 1)
            plan.append((t, g, k, t == first, t == last))
    return plan

PLAN = conv_plan()
NMM = len(PLAN)
NMM_TOT = NMM + NCH  # plus the half-patch tiles
ROWS = [16 * 8, 16 * 4]  # used psum rows per full group


def build_program():
    nc = bacc.Bacc("TRN2")
    x_d = nc.dram_tensor("x", [128, NCF * B + NCH * HB], BF16,
                         kind="ExternalInput")
    # full w tiles, then half-patch w tiles, then 3 decoder groups
    w_d = nc.dram_tensor("w", [128, NMM_TOT * 32 + 3 * OUT], BF16,
                         kind="ExternalInput")
    b_d = nc.dram_tensor("bias", [128, 3], F32, kind="ExternalInput")
    o_d = nc.dram_tensor("out", [OUT, B], F32, kind="ExternalOutput")
    o2_d = nc.dram_tensor("out2", [OUT, HB], F32, kind="ExternalOutput")

    with tile.TileContext(nc) as tc:
        with (
            tc.tile_pool(name="const", bufs=1) as constp,
            tc.tile_pool(name="yps", bufs=2, space="PSUM") as ypsp,
            tc.tile_pool(name="hps", bufs=1, space="PSUM") as hpsp,
            tc.tile_pool(name="wps", bufs=1, space="PSUM") as wpsp,
            tc.tile_pool(name="ops", bufs=1, space="PSUM") as opsp,
        ):
            w_sb = constp.tile([128, NMM_TOT * 32 + 3 * OUT], BF16)
            nc.scalar.dma_start(out=w_sb[:], in_=w_d[:])
            dec_sb = w_sb[:, NMM_TOT * 32:]
            bias_sb = constp.tile([128, 3], F32)
            nc.scalar.dma_start(out=bias_sb[:], in_=b_d[:])

            xt = constp.tile([128, NCF, B], BF16)
            for c0, n in DMA_SPLITS:
                nc.sync.dma_start(
                    out=xt[:, c0:c0 + n, :],
                    in_=x_d[:, c0 * B:(c0 + n) * B],
                )
            xh = constp.tile([128, NCH, HB], BF16)
            nc.sync.dma_start(
                out=xh[:],
                in_=x_d[:, NCF * B:NCF * B + NCH * HB],
            )

            # PE clock warm-up during the DMA pipe-fill window: M=128 dummy
            # matmuls on a memset scratch tile (output never read)
            warm_sb = constp.tile([128, 128 + B], BF16)
            nc.gpsimd.memset(warm_sb[:], 0.0)
            warm_ps = wpsp.tile([128, B], F32)
            for _ in range(9):
                nc.tensor.matmul(
                    warm_ps[:],
                    warm_sb[:, 0:128],
                    warm_sb[:, 128:],
                    start=True,
                    stop=True,
                )

            y_sb = constp.tile([128, 2, B], BF16)
            yh_sb = constp.tile([16, HB], BF16)
            ps = [ypsp.tile([128, B], F32, name=f"ps{g}") for g in range(2)]
            ps_h = hpsp.tile([32, HB], F32)
            out_ps = opsp.tile([OUT, B], F32)
            out_ps_h = hpsp.tile([OUT, HB], F32)
            out_sb = constp.tile([OUT, B], F32)
            out_h_sb = constp.tile([OUT, HB], F32)

            def conv_mm(i, t, g, k, st, sp):
                nc.tensor.matmul(
                    ps[g][32 * k:32 * k + 32, :],
                    w_sb[:, 32 * i:32 * i + 32],
                    xt[:, t, :],
                    start=st,
                    stop=sp,
                    tile_position=(0, 32 * k),
                )

            ga = [e for e in enumerate(PLAN) if e[1][1] == 0]
            gb = [e for e in enumerate(PLAN) if e[1][1] == 1]
            for i, (t, _, k, st, sp) in ga:
                conv_mm(i, t, 0, k, st, sp)
            nc.scalar.activation(
                out=y_sb[0:ROWS[0], 0, :],
                in_=ps[0][0:ROWS[0], :],
                func=mybir.ActivationFunctionType.Relu,
                bias=bias_sb[0:ROWS[0], 0:1],
            )
            # a few group-B matmuls before the group-A decode so the PE
            # FIFO doesn't stall waiting on the relu
            for i, (t, _, k, st, sp) in gb[:8]:
                conv_mm(i, t, 1, k, st, sp)
            nc.tensor.matmul(
                out_ps[:],
                dec_sb[0:ROWS[0], 0:OUT],
                y_sb[0:ROWS[0], 0, :],
                start=True,
                stop=False,
            )
            for i, (t, _, k, st, sp) in gb[8:]:
                conv_mm(i, t, 1, k, st, sp)
            nc.scalar.activation(
                out=y_sb[0:ROWS[1], 1, :],
                in_=ps[1][0:ROWS[1], :],
                func=mybir.ActivationFunctionType.Relu,
                bias=bias_sb[0:ROWS[1], 1:2],
            )
            # half patch: 7 N=256 chunks into its own psum, then its decode
            # lands only in this core's image-half columns of out_ps
            for th in range(NCH):
                nc.tensor.matmul(
                    ps_h[:],
                    w_sb[:, 32 * (NMM + th):32 * (NMM + th) + 32],
                    xh[:, th, :],
                    start=(th == 0),
                    stop=(th == NCH - 1),
                )
            nc.scalar.activation(
                out=yh_sb[:],
                in_=ps_h[0:16, :],
                func=mybir.ActivationFunctionType.Relu,
                bias=bias_sb[0:16, 2:3],
            )
            nc.tensor.matmul(
                out_ps[:],
                dec_sb[0:ROWS[1], OUT:2 * OUT],
                y_sb[0:ROWS[1], 1, :],
                start=False,
                stop=True,
            )
            nc.tensor.matmul(
                out_ps_h[:],
                dec_sb[0:16, 2 * OUT:3 * OUT],
                yh_sb[:],
                start=True,
                stop=True,
            )
            nc.vector.tensor_copy(out_sb[:], out_ps[:])
            nc.sync.dma_start(out=o_d[:], in_=out_sb[:])
            nc.scalar.activation(
                out=out_h_sb[:],
                in_=out_ps_h[:],
                func=mybir.ActivationFunctionType.Copy,
            )
            nc.scalar.dma_start(out=o2_d[:], in_=out_h_sb[:])

    return nc


def stage_core(core, x_pm, weight, bias, dec_w):
    """Host-side staging for one core. x_pm: (B, 100, 784) float32."""
    import ml_dtypes

    p0 = NPF * core
    pids = list(range(p0, p0 + NPF))
    pid_h = 96 + core // 2
    img_off = (core % 2) * HB

    xs = np.zeros((B, NCF * 128), np.float32)
    xs[:, :NPF * PPX] = x_pm[:, p0:p0 + NPF, :].reshape(B, NPF * PPX)
    xs = np.ascontiguousarray(
        xs.reshape(B, NCF, 128).transpose(2, 1, 0)
    ).reshape(128, NCF * B)
    xhs = np.zeros((HB, NCH * 128), np.float32)
    xhs[:, :PPX] = x_pm[img_off:img_off + HB, pid_h, :]
    xhs = np.ascontiguousarray(
        xhs.reshape(HB, NCH, 128).transpose(2, 1, 0)
    ).reshape(128, NCH * HB)
    x_all = np.concatenate([xs, xhs], axis=1).astype(ml_dtypes.bfloat16)

    wr = np.asarray(weight, np.float32).reshape(F, P, PPX)
    w_big = np.zeros((128, NMM_TOT * 32), np.float32)
    for i, (t, g, k, _, _) in enumerate(PLAN):
        for r in range(128):
            px = 128 * t + r
            p = px // PPX
            if p >= NPF:
                continue
            pl = p - 8 * g
            if pl < 0 or pl // 2 != k:
                continue
            q = px % PPX
            w_big[r, 32 * i + (pl % 2) * 16:32 * i + (pl % 2) * 16 + F] = \
                wr[:, p0 + p, q]
    for th in range(NCH):
        for r in range(128):
            q = 128 * th + r
            if q >= PPX:
                continue
            i = NMM + th
            w_big[r, 32 * i:32 * i + F] = wr[:, pid_h, q]

    br = np.asarray(bias, np.float32).reshape(F, P)
    dr = np.asarray(dec_w, np.float32).reshape(OUT, F, P)
    b_st = np.zeros((128, 3), np.float32)
    d_st = np.zeros((128, 3 * OUT), np.float32)
    for p in range(NPF):
        g, pl = p // 8, p % 8
        j = 16 * pl + np.arange(F)
        b_st[j, g] = br[:, pids[p]]
        d_st[j[:, None], g * OUT + np.arange(OUT)[None, :]] = \
            dr[:, :, pids[p]].T
    j = np.arange(F)
    b_st[j, 2] = br[:, pid_h]
    d_st[j[:, None], 2 * OUT + np.arange(OUT)[None, :]] = dr[:, :, pid_h].T

    w_all = np.concatenate([w_big, d_st], axis=1).astype(ml_dtypes.bfloat16)
    return {"x": x_all, "w": w_all, "bias": b_st}


_cache = {}


def _get_nc():
    if "nc" not in _cache:
        nc = build_program()
        nc.finalize()
        _cache["nc"] = nc
    return _cache["nc"]


def make_in_maps(x, weight, bias, dec_w):
    x = np.asarray(x, np.float32)
    # patch-major pixel order: (b, ph, pw, k, l)
    x_pm = np.ascontiguousarray(
        x.reshape(B, 10, 28, 10, 28).transpose(0, 1, 3, 2, 4)
    ).reshape(B, P, PPX)
    return [stage_core(c, x_pm, weight, bias, dec_w) for c in range(NCORES)]


def combine(results, dec_b):
    acc = np.zeros((OUT, B), np.float32)
    for core, r in enumerate(results):
        acc += r["out"]
        off = (core % 2) * HB
        acc[:, off:off + HB] += r["out2"]
    return acc.T + np.asarray(dec_b, np.float32)


def _install_ntff_hook():
    """Provide the missing antenv.axon_hooks module so trace=True works
    under axon (replicates trn_boot._ntff_profile_via_ctypes)."""
    import contextlib
    import ctypes
    import types

    if "antenv.axon_hooks" in sys.modules:
        return
    so_path = "/opt/axon/libaxon_pjrt.so"
    holder = {}
    mod = types.ModuleType("antenv.axon_hooks")
    mod.set_axon_ntff_profile_hook = lambda h: holder.__setitem__("h", h)
    mod.get_axon_ntff_profile_hook = lambda: holder.get("h")
    sys.modules["antenv.axon_hooks"] = mod
    try:
        import antenv
        antenv.axon_hooks = mod
    except ImportError:
        pass

    lib = ctypes.CDLL(so_path)
    if not hasattr(lib, "axon_start_nrt_profile"):
        return
    lib.axon_start_nrt_profile.argtypes = [
        ctypes.POINTER(ctypes.c_int64), ctypes.c_size_t]
    lib.axon_start_nrt_profile.restype = ctypes.c_int64
    lib.axon_stop_nrt_profile.argtypes = [ctypes.c_char_p]
    lib.axon_stop_nrt_profile.restype = ctypes.c_int64

    @contextlib.contextmanager
    def _hook(output_dir, device_ids):
        import jax
        jax.devices()
        if device_ids:
            ids = (ctypes.c_int64 * len(device_ids))(*device_ids)
            rc = lib.axon_start_nrt_profile(ids, len(device_ids))
        else:
            rc = lib.axon_start_nrt_profile(None, 0)
        if rc != 0:
            raise RuntimeError(f"axon_start_nrt_profile rc={rc}")
        try:
            yield
        finally:
            n = lib.axon_stop_nrt_profile(str(output_dir).encode())
            print(f"profile: {n} file(s) written to {output_dir}")

    mod.set_axon_ntff_profile_hook(_hook)


def run(x, weight, bias, dec_w, dec_b, trace=False):
    from concourse import bass_utils
    from concourse.bass_utils import run_bass_kernel_spmd

    if trace:
        _install_ntff_hook()
        bass_utils.upload_artifacts = lambda tmpdir: tmpdir

    nc = _get_nc()
    in_maps = make_in_maps(x, weight, bias, dec_w)
    r = run_bass_kernel_spmd(nc, in_maps, list(range(NCORES)), trace=trace)
    return combine(r.results, dec_b), r


def kernel(x, weight, bias, dec_w, dec_b):
    out, _ = run(x, weight, bias, dec_w, dec_b, trace=False)
    return out


# revision 38
# speedup vs baseline: 1.1418x; 1.0145x over previous
"""Trainium2 Bass kernel for nn_LCN (locally-connected network).

Computation:
  x: (512, 1, 280, 280) -> non-overlapping 28x28 patches (10x10 grid, P=100)
  y[b, f, p] = sum_q x[b, p, q] * w[f*100+p, q]    (q = k*28+l, 784 per patch)
  y = relu(y + bias[f*100+p]);  out = y_flat @ dec_w.T + dec_b  (j = f*100+p)

Sharding: patch-parallel, perfectly balanced. All cores run the same
program on 12 full patches (all 512 images) plus one HALF patch (one
256-image half of a shared patch; patches 96-99 are each split between two
cores). Per core:
  - host stages x TRANSPOSED as xT [128 px, chunk, batch] bf16 (im2col +
    transpose + cast on host; DMA reads are contiguous multi-KB runs per
    partition at full HBM bandwidth)
  - conv: one matmul per (128-px chunk, patch-pair 32-col PSUM window),
    K=128 N=512 bf16 accumulating per patch group; the half patch uses
    N=256 chunks into its own small PSUM tile
  - ACT: relu(psum + bias) -> y_sb (bf16)
  - decoder: accumulating matmuls into one [10, 512] PSUM tile; the half
    patch adds only into its 256 image columns
Host sums the 8 per-core partial decoder outputs and adds dec_b.
"""

import sys

import numpy as np

for _p in ("/opt/trn_rl_repo", "/opt/trn_rl_repo/concourse"):
    if _p not in sys.path:
        sys.path.insert(0, _p)

import concourse.mybir as mybir
import concourse.tile as tile
from concourse import bacc

F32 = mybir.dt.float32
BF16 = mybir.dt.bfloat16

# Problem constants
B = 512
HB = 256             # half batch (shared-patch image half)
P = 100
F = 16
OUT = 10
PPX = 784            # pixels per patch (28*28)
NCORES = 8

NPF = 12             # full patches per core
NCF = 74             # full-region chunks: ceil(12*784/128) (64 px end pad)
NCH = 7              # half-region chunks: ceil(784/128) (112 px end pad)
# full-region x splits; the half region is one final split
_SPLIT_SIZES = [4, 8, 12, 12, 12, 12, 8, 6]
DMA_SPLITS = []
_c = 0
for _s in _SPLIT_SIZES:
    DMA_SPLITS.append((_c, _s))
    _c += _s
assert _c == NCF


def conv_plan():
    """Full-region matmul plan: one entry per (chunk, patch-pair window)."""
    plan = []
    for t in range(NCF):
        p0 = min((128 * t) // PPX, NPF - 1)
        p1 = min((128 * t + 127) // PPX, NPF - 1)
        g = p0 // 8
        pairs = sorted({(p - 8 * g) // 2 for p in (p0, p1)})
        for k in pairs:
            first = ((8 * g + 2 * k) * PPX) // 128
            last = min(((8 * g + 2 * k + 2) * PPX - 1) // 128, NCF - 1)
            plan.append((t, g, k, t == first, t == last))
    return plan

PLAN = conv_plan()
NMM = len(PLAN)
NMM_TOT = NMM + NCH  # plus the half-patch tiles
ROWS = [16 * 8, 16 * 4]  # used psum rows per full group


def build_program():
    nc = bacc.Bacc("TRN2")
    x_d = nc.dram_tensor("x", [128, NCF * B + NCH * HB], BF16,
                         kind="ExternalInput")
    # full w tiles, then half-patch w tiles, then 3 decoder groups
    w_d = nc.dram_tensor("w", [128, NMM_TOT * 32 + 3 * OUT], BF16,
                         kind="ExternalInput")
    b_d = nc.dram_tensor("bias", [128, 3], F32, kind="ExternalInput")
    o_d = nc.dram_tensor("out", [OUT, B], F32, kind="ExternalOutput")
    o2_d = nc.dram_tensor("out2", [OUT, HB], F32, kind="ExternalOutput")

    with tile.TileContext(nc) as tc:
        with (
            tc.tile_pool(name="const", bufs=1) as constp,
            tc.tile_pool(name="yps", bufs=2, space="PSUM") as ypsp,
            tc.tile_pool(name="hps", bufs=1, space="PSUM") as hpsp,
            tc.tile_pool(name="wps", bufs=1, space="PSUM") as wpsp,
            tc.tile_pool(name="ops", bufs=1, space="PSUM") as opsp,
        ):
            w_sb = constp.tile([128, NMM_TOT * 32 + 3 * OUT], BF16)
            nc.scalar.dma_start(out=w_sb[:], in_=w_d[:])
            dec_sb = w_sb[:, NMM_TOT * 32:]
            bias_sb = constp.tile([128, 3], F32)
            nc.scalar.dma_start(out=bias_sb[:], in_=b_d[:])

            xh = constp.tile([128, NCH, HB], BF16)
            nc.sync.dma_start(
                out=xh[:],
                in_=x_d[:, NCF * B:NCF * B + NCH * HB],
            )
            xt = constp.tile([128, NCF, B], BF16)
            for c0, n in DMA_SPLITS:
                nc.sync.dma_start(
                    out=xt[:, c0:c0 + n, :],
                    in_=x_d[:, c0 * B:(c0 + n) * B],
                )

            # PE clock warm-up during the DMA pipe-fill window: M=128 dummy
            # matmuls on a memset scratch tile (output never read)
            warm_sb = constp.tile([128, 128 + B], BF16)
            nc.gpsimd.memset(warm_sb[:], 0.0)
            warm_ps = wpsp.tile([128, B], F32)
            for _ in range(9):
                nc.tensor.matmul(
                    warm_ps[:],
                    warm_sb[:, 0:128],
                    warm_sb[:, 128:],
                    start=True,
                    stop=True,
                )

            y_sb = constp.tile([128, 2, B], BF16)
            yh_sb = constp.tile([16, HB], BF16)
            ps = [ypsp.tile([128, B], F32, name=f"ps{g}") for g in range(2)]
            ps_h = hpsp.tile([32, HB], F32)
            out_ps = opsp.tile([OUT, B], F32)
            out_ps_h = hpsp.tile([OUT, HB], F32)
            out_sb = constp.tile([OUT, B], F32)
            out_h_sb = constp.tile([OUT, HB], F32)

            def conv_mm(i, t, g, k, st, sp):
                nc.tensor.matmul(
                    ps[g][32 * k:32 * k + 32, :],
                    w_sb[:, 32 * i:32 * i + 32],
                    xt[:, t, :],
                    start=st,
                    stop=sp,
                    tile_position=(0, 32 * k),
                )

            # half-patch pipeline runs in the early DMA pipe-fill
            # bubble: its data is tiny and issued first, and its decode
            # uses an independent PSUM tile
            for th in range(NCH):
                nc.tensor.matmul(
                    ps_h[:],
                    w_sb[:, 32 * (NMM + th):32 * (NMM + th) + 32],
                    xh[:, th, :],
                    start=(th == 0),
                    stop=(th == NCH - 1),
                )
            nc.scalar.activation(
                out=yh_sb[:],
                in_=ps_h[0:16, :],
                func=mybir.ActivationFunctionType.Relu,
                bias=bias_sb[0:16, 2:3],
            )
            nc.tensor.matmul(
                out_ps_h[:],
                dec_sb[0:16, 2 * OUT:3 * OUT],
                yh_sb[:],
                start=True,
                stop=True,
            )
            nc.scalar.activation(
                out=out_h_sb[:],
                in_=out_ps_h[:],
                func=mybir.ActivationFunctionType.Copy,
            )
            nc.scalar.dma_start(out=o2_d[:], in_=out_h_sb[:])

            ga = [e for e in enumerate(PLAN) if e[1][1] == 0]
            gb = [e for e in enumerate(PLAN) if e[1][1] == 1]
            for i, (t, _, k, st, sp) in ga:
                conv_mm(i, t, 0, k, st, sp)
            nc.scalar.activation(
                out=y_sb[0:ROWS[0], 0, :],
                in_=ps[0][0:ROWS[0], :],
                func=mybir.ActivationFunctionType.Relu,
                bias=bias_sb[0:ROWS[0], 0:1],
            )
            # a few group-B matmuls before the group-A decode so the PE
            # FIFO doesn't stall waiting on the relu
            for i, (t, _, k, st, sp) in gb[:8]:
                conv_mm(i, t, 1, k, st, sp)
            nc.tensor.matmul(
                out_ps[:],
                dec_sb[0:ROWS[0], 0:OUT],
                y_sb[0:ROWS[0], 0, :],
                start=True,
                stop=False,
            )
            for i, (t, _, k, st, sp) in gb[8:]:
                conv_mm(i, t, 1, k, st, sp)
            nc.scalar.activation(
                out=y_sb[0:ROWS[1], 1, :],
                in_=ps[1][0:ROWS[1], :],
                func=mybir.ActivationFunctionType.Relu,
                bias=bias_sb[0:ROWS[1], 1:2],
            )
            nc.tensor.matmul(
                out_ps[:],
                dec_sb[0:ROWS[1], OUT:2 * OUT],
                y_sb[0:ROWS[1], 1, :],
                start=False,
                stop=True,
            )
            nc.vector.tensor_copy(out_sb[:], out_ps[:])
            nc.sync.dma_start(out=o_d[:], in_=out_sb[:])


    return nc


def stage_core(core, x_pm, weight, bias, dec_w):
    """Host-side staging for one core. x_pm: (B, 100, 784) float32."""
    import ml_dtypes

    p0 = NPF * core
    pids = list(range(p0, p0 + NPF))
    pid_h = 96 + core // 2
    img_off = (core % 2) * HB

    xs = np.zeros((B, NCF * 128), np.float32)
    xs[:, :NPF * PPX] = x_pm[:, p0:p0 + NPF, :].reshape(B, NPF * PPX)
    xs = np.ascontiguousarray(
        xs.reshape(B, NCF, 128).transpose(2, 1, 0)
    ).reshape(128, NCF * B)
    xhs = np.zeros((HB, NCH * 128), np.float32)
    xhs[:, :PPX] = x_pm[img_off:img_off + HB, pid_h, :]
    xhs = np.ascontiguousarray(
        xhs.reshape(HB, NCH, 128).transpose(2, 1, 0)
    ).reshape(128, NCH * HB)
    x_all = np.concatenate([xs, xhs], axis=1).astype(ml_dtypes.bfloat16)

    wr = np.asarray(weight, np.float32).reshape(F, P, PPX)
    w_big = np.zeros((128, NMM_TOT * 32), np.float32)
    for i, (t, g, k, _, _) in enumerate(PLAN):
        for r in range(128):
            px = 128 * t + r
            p = px // PPX
            if p >= NPF:
                continue
            pl = p - 8 * g
            if pl < 0 or pl // 2 != k:
                continue
            q = px % PPX
            w_big[r, 32 * i + (pl % 2) * 16:32 * i + (pl % 2) * 16 + F] = \
                wr[:, p0 + p, q]
    for th in range(NCH):
        for r in range(128):
            q = 128 * th + r
            if q >= PPX:
                continue
            i = NMM + th
            w_big[r, 32 * i:32 * i + F] = wr[:, pid_h, q]

    br = np.asarray(bias, np.float32).reshape(F, P)
    dr = np.asarray(dec_w, np.float32).reshape(OUT, F, P)
    b_st = np.zeros((128, 3), np.float32)
    d_st = np.zeros((128, 3 * OUT), np.float32)
    for p in range(NPF):
        g, pl = p // 8, p % 8
        j = 16 * pl + np.arange(F)
        b_st[j, g] = br[:, pids[p]]
        d_st[j[:, None], g * OUT + np.arange(OUT)[None, :]] = \
            dr[:, :, pids[p]].T
    j = np.arange(F)
    b_st[j, 2] = br[:, pid_h]
    d_st[j[:, None], 2 * OUT + np.arange(OUT)[None, :]] = dr[:, :, pid_h].T

    w_all = np.concatenate([w_big, d_st], axis=1).astype(ml_dtypes.bfloat16)
    return {"x": x_all, "w": w_all, "bias": b_st}


_cache = {}


def _get_nc():
    if "nc" not in _cache:
        nc = build_program()
        nc.finalize()
        _cache["nc"] = nc
    return _cache["nc"]


def make_in_maps(x, weight, bias, dec_w):
    x = np.asarray(x, np.float32)
    # patch-major pixel order: (b, ph, pw, k, l)
    x_pm = np.ascontiguousarray(
        x.reshape(B, 10, 28, 10, 28).transpose(0, 1, 3, 2, 4)
    ).reshape(B, P, PPX)
    return [stage_core(c, x_pm, weight, bias, dec_w) for c in range(NCORES)]


def combine(results, dec_b):
    acc = np.zeros((OUT, B), np.float32)
    for core, r in enumerate(results):
        acc += r["out"]
        off = (core % 2) * HB
        acc[:, off:off + HB] += r["out2"]
    return acc.T + np.asarray(dec_b, np.float32)


def _install_ntff_hook():
    """Provide the missing antenv.axon_hooks module so trace=True works
    under axon (replicates trn_boot._ntff_profile_via_ctypes)."""
    import contextlib
    import ctypes
    import types

    if "antenv.axon_hooks" in sys.modules:
        return
    so_path = "/opt/axon/libaxon_pjrt.so"
    holder = {}
    mod = types.ModuleType("antenv.axon_hooks")
    mod.set_axon_ntff_profile_hook = lambda h: holder.__setitem__("h", h)
    mod.get_axon_ntff_profile_hook = lambda: holder.get("h")
    sys.modules["antenv.axon_hooks"] = mod
    try:
        import antenv
        antenv.axon_hooks = mod
    except ImportError:
        pass

    lib = ctypes.CDLL(so_path)
    if not hasattr(lib, "axon_start_nrt_profile"):
        return
    lib.axon_start_nrt_profile.argtypes = [
        ctypes.POINTER(ctypes.c_int64), ctypes.c_size_t]
    lib.axon_start_nrt_profile.restype = ctypes.c_int64
    lib.axon_stop_nrt_profile.argtypes = [ctypes.c_char_p]
    lib.axon_stop_nrt_profile.restype = ctypes.c_int64

    @contextlib.contextmanager
    def _hook(output_dir, device_ids):
        import jax
        jax.devices()
        if device_ids:
            ids = (ctypes.c_int64 * len(device_ids))(*device_ids)
            rc = lib.axon_start_nrt_profile(ids, len(device_ids))
        else:
            rc = lib.axon_start_nrt_profile(None, 0)
        if rc != 0:
            raise RuntimeError(f"axon_start_nrt_profile rc={rc}")
        try:
            yield
        finally:
            n = lib.axon_stop_nrt_profile(str(output_dir).encode())
            print(f"profile: {n} file(s) written to {output_dir}")

    mod.set_axon_ntff_profile_hook(_hook)


def run(x, weight, bias, dec_w, dec_b, trace=False):
    from concourse import bass_utils
    from concourse.bass_utils import run_bass_kernel_spmd

    if trace:
        _install_ntff_hook()
        bass_utils.upload_artifacts = lambda tmpdir: tmpdir

    nc = _get_nc()
    in_maps = make_in_maps(x, weight, bias, dec_w)
    r = run_bass_kernel_spmd(nc, in_maps, list(range(NCORES)), trace=trace)
    return combine(r.results, dec_b), r


def kernel(x, weight, bias, dec_w, dec_b):
    out, _ = run(x, weight, bias, dec_w, dec_b, trace=False)
    return out
